# revision 15
# baseline (speedup 1.0000x reference)
"""Multi-head attention Trainium2 Bass kernel.

Problem: nn_MultiHeadAttention (B=2, L=2048, d_model=1024, H=16, d_k=64),
returns (out, attn) where attn [B,H,L,L] is itself an output.

Sharding: 8 cores = (batch b in {0,1}) x (head-group g in {0..3}); each core
computes 4 heads of one batch.  Tensor-parallel on W_qkv columns / W_o rows,
data-parallel on batch.  The o_proj partial sums are reduced on the host
(4 partials per batch); attn comes back transposed per head (attnT[h] = A.T)
because the on-device layout keeps keys on partitions.

Device dataflow (all transposed / "layout 2"):
  qT,kT [256,L]   = (x @ Wq/Wk + b)^T   (feature d on partitions, 2 heads/tile)
  v1    [L,4*65]  =  x @ Wv + b, with a ones-column appended per head
  S^T   [128j,ib] = kT_h-slice.T @ qT_h   (K = d_k = 64; head pairs packed
                    into PE row-groups via tile_position (0,0)/(64,0))
  E^T   = exp(S^T/8)                      (ScalarE, scale folds 1/sqrt(64))
  ctxU^T,Sigma    = v1_h.T @ E^T          (ones column makes the softmax
                                           denominator fall out of row 64)
  r = 1/Sigma, R = ones x r (PE outer-product broadcast across partitions)
  A^T = E^T * R  -> DMA to attnT HBM
  ctx^T = ctxU^T * R ; out = ctx^T.T @ W_o  (interleaved per i-block)
"""

import sys

if "/opt/trn_rl_repo" not in sys.path:
    sys.path.insert(0, "/opt/trn_rl_repo")

import numpy as np

import concourse.bass as bass
import concourse.bacc as bacc
import concourse.mybir as mybir
from concourse.tile import TileContext

FP = mybir.dt.float32
BF = mybir.dt.bfloat16
P = 128

L_FULL = 2048
D_FULL = 1024
H_TOTAL = 16
DK = 64
N_CORES = 8
HPC = 4  # heads per core


def build_mha_core(L=L_FULL, D=D_FULL, reps=1, bf16_qk=False, bf16_x=False,
                   gps_chunks=0, skip_attn_write=False, skip_oproj=False,
                   fused_oproj=True, pack_scores=True):
    """Build the per-core Bass program (SPMD; all cores run the same code)."""
    KT = D // P              # k-tiles over d_model
    NJT = L // P             # key tiles (128 keys each)
    IB = min(512, L)         # query block (free dim of S^T tiles)
    NIB = L // IB
    MW = HPC * DK // P       # 2 partition tiles of qT/kT (2 heads each)
    VW = HPC * DK            # 256: per-core feature width
    ACC = 2                  # key-tiles per A-chunk (DVE/DMA granularity)
    QKT = BF if bf16_qk else FP
    XT = BF if bf16_x else FP

    nc = bacc.Bacc("TRN2", target_bir_lowering=False, debug=False)

    xT_d = nc.declare_dram_parameter("xT", [D, L], FP, isOutput=False)
    wq_d = nc.declare_dram_parameter("wq", [D, VW], FP, isOutput=False)
    wk_d = nc.declare_dram_parameter("wk", [D, VW], FP, isOutput=False)
    wv_d = nc.declare_dram_parameter("wv", [D, VW], FP, isOutput=False)
    bq_d = nc.declare_dram_parameter("bq", [1, VW], FP, isOutput=False)
    bk_d = nc.declare_dram_parameter("bk", [1, VW], FP, isOutput=False)
    bv_d = nc.declare_dram_parameter("bv", [1, VW], FP, isOutput=False)
    wo_d = nc.declare_dram_parameter("wo", [VW, D], FP, isOutput=False)
    ones_d = nc.declare_dram_parameter("ones", [1, max(L, P)], FP, isOutput=False)

    attnT_d = nc.declare_dram_parameter("attnT", [HPC, L, L], FP, isOutput=True)
    outp_d = nc.declare_dram_parameter("outp", [L, D], FP, isOutput=True)

    from contextlib import ExitStack

    with TileContext(nc) as tc, ExitStack() as ctx:
        consts = ctx.enter_context(tc.tile_pool(name="consts", bufs=1))
        persist = ctx.enter_context(tc.tile_pool(name="persist", bufs=1))

        ones_sb = consts.tile([1, max(L, P)], FP, tag="ones", name="ones_sb")
        nc.sync.dma_start(ones_sb, ones_d.ap())
        bq_sb = consts.tile([1, VW], FP, tag="bq", name="bq_sb")
        nc.sync.dma_start(bq_sb, bq_d.ap())
        bk_sb = consts.tile([1, VW], FP, tag="bk", name="bk_sb")
        nc.sync.dma_start(bk_sb, bk_d.ap())
        bv_sb = consts.tile([1, VW], FP, tag="bv", name="bv_sb")
        nc.sync.dma_start(bv_sb, bv_d.ap())
        wo_sb = consts.tile([P, VW // P, D], FP, tag="wo", name="wo_sb")
        nc.sync.dma_start(wo_sb, wo_d.ap().rearrange("(t p) c -> p t c", p=P))

        qT = persist.tile([P, MW, L], QKT, tag="qT", name="qT")
        kT = persist.tile([P, MW, L], QKT, tag="kT", name="kT")
        v1 = persist.tile([P, NJT, HPC * (DK + 1)], FP, tag="v1", name="v1")
        ctxT = persist.tile([P, MW, L], FP, tag="ctxT", name="ctxT")

        for _rep in range(reps):
            # ---------------- phase 1: projections ----------------
            with (
                tc.tile_pool(name=f"proj_sb_{_rep}", bufs=1) as proj_sb,
                tc.tile_pool(name=f"proj_ps_{_rep}", bufs=2, space="PSUM") as proj_ps,
            ):
                dma_x = nc.gpsimd.dma_start if bf16_x else nc.sync.dma_start
                xt = proj_sb.tile([P, KT, L], XT, tag="xt", name="xt")
                wq_sb = proj_sb.tile([P, KT, VW], XT, tag="wq", name="wq_sb")
                wk_sb = proj_sb.tile([P, KT, VW], XT, tag="wk", name="wk_sb")
                wv_sb = proj_sb.tile([P, KT, VW], XT, tag="wv", name="wv_sb")
                # per-k-tile loads so the first matmuls start early
                for kt in range(KT):
                    sl = slice(kt * P, (kt + 1) * P)
                    dma_x(wq_sb[:, kt], wq_d.ap()[sl])
                    dma_x(wk_sb[:, kt], wk_d.ap()[sl])
                    dma_x(wv_sb[:, kt], wv_d.ap()[sl])
                    dma_x(xt[:, kt], xT_d.ap()[sl])

                # ones columns of v1 (head-local col 64 of each 65-block)
                v1_ones = v1.rearrange("p jt (h e) -> p jt h e", h=HPC)[
                    :, :, :, DK : DK + 1
                ]
                nc.vector.memset(v1_ones, 1.0)

                # qT / kT:  psum[j_feat, i] += W-tile.T @ xT[d, i]
                for w_sb, b_sb, dst in ((wq_sb, bq_sb, qT), (wk_sb, bk_sb, kT)):
                    for mt in range(MW):
                        for nb in range(NIB):
                            ps = proj_ps.tile([P, IB], FP, tag="pj", name="pj_ps")
                            for kt in range(KT):
                                nc.tensor.matmul(
                                    ps,
                                    w_sb[:, kt, mt * P : (mt + 1) * P],
                                    xt[:, kt, nb * IB : (nb + 1) * IB],
                                    start=(kt == 0),
                                    stop=False,
                                )
                            nc.tensor.matmul(
                                ps,
                                b_sb[:1, mt * P : (mt + 1) * P],
                                ones_sb[:1, nb * IB : (nb + 1) * IB],
                                start=False,
                                stop=True,
                            )
                            nc.scalar.copy(dst[:, mt, nb * IB : (nb + 1) * IB], ps)

                # v (natural layout):  psum[j_tok, d] += xT-tile.T @ Wv-tile
                for jt in range(NJT):
                    ps = proj_ps.tile([P, VW], FP, tag="pjv", name="pjv_ps")
                    for kt in range(KT):
                        nc.tensor.matmul(
                            ps,
                            xt[:, kt, jt * P : (jt + 1) * P],
                            wv_sb[:, kt, :],
                            start=(kt == 0),
                            stop=False,
                        )
                    nc.tensor.matmul(
                        ps, ones_sb[:1, :P], bv_sb, start=False, stop=True
                    )
                    v1_dst = v1.rearrange("p jt (h e) -> p jt h e", h=HPC)[
                        :, jt, :, 0:DK
                    ]
                    nc.scalar.copy(v1_dst, ps.rearrange("p (h e) -> p h e", h=HPC))

            # ------------- phase 2+3: attention with interleaved o_proj -----
            with (
                tc.tile_pool(name=f"att_sb_{_rep}", bufs=1) as att_sb,
                tc.tile_pool(name=f"att_ps_{_rep}", bufs=2, space="PSUM") as att_ps,
                tc.tile_pool(name=f"ctx_ps_{_rep}", bufs=2, space="PSUM") as ctx_ps,
                tc.tile_pool(name=f"op_sb_{_rep}", bufs=2) as op_sb,
                tc.tile_pool(name=f"op_ps_{_rep}", bufs=2, space="PSUM") as op_ps,
            ):
                def oproj(it_range):
                    CB = min(512, D)
                    for it in it_range:
                        for cb in range(D // CB):
                            ps = op_ps.tile([P, CB], FP, tag="op", name="op_ps_t")
                            for t in range(MW):
                                nc.tensor.matmul(
                                    ps,
                                    ctxT[:, t, it * P : (it + 1) * P],
                                    wo_sb[:, t, cb * CB : (cb + 1) * CB],
                                    start=(t == 0),
                                    stop=(t == MW - 1),
                                )
                            ob = op_sb.tile([P, CB], FP, tag="ob", name="ob")
                            nc.scalar.copy(ob, ps)
                            nc.sync.dma_start(
                                outp_d.ap()[
                                    it * P : (it + 1) * P, cb * CB : (cb + 1) * CB
                                ],
                                ob,
                            )

                IT_PER_IB = L // P // NIB
                for ib in range(NIB):
                    for pt in range(MW):
                        h_pair = (2 * pt, 2 * pt + 1)
                        E_pair, cx_pair = [], []
                        for u, h in enumerate(h_pair):
                            E_pair.append(
                                att_sb.tile(
                                    [P, NJT, IB], FP, tag="E", bufs=3,
                                    name=f"E_{h}_{ib}",
                                )
                            )
                            cx_pair.append(
                                ctx_ps.tile(
                                    [DK + 1, IB], FP, tag="cx", name=f"cx_{h}_{ib}"
                                )
                            )
                        # scores + exp, two key-tiles per exp op (FD = 2*IB)
                        for jtp in range((NJT + 1) // 2):
                            jts = [jt for jt in (2 * jtp, 2 * jtp + 1) if jt < NJT]
                            S_pair = [
                                att_ps.tile(
                                    [P, len(jts), IB], FP, tag="S",
                                    name=f"S_{h}_{jtp}",
                                )
                                for h in h_pair
                            ]
                            for u, h in enumerate(h_pair):
                                lo = u * DK
                                tp = (lo, 0) if pack_scores else None
                                for q, jt in enumerate(jts):
                                    nc.tensor.matmul(
                                        S_pair[u][:, q],
                                        kT[lo : lo + DK, pt, jt * P : (jt + 1) * P],
                                        qT[lo : lo + DK, pt, ib * IB : (ib + 1) * IB],
                                        start=True,
                                        stop=True,
                                        tile_position=tp,
                                    )
                            for u, h in enumerate(h_pair):
                                nc.scalar.activation(
                                    E_pair[u][:, 2 * jtp : 2 * jtp + len(jts)],
                                    S_pair[u],
                                    mybir.ActivationFunctionType.Exp,
                                    scale=float(1.0 / np.sqrt(DK)),
                                )
                            for u, h in enumerate(h_pair):
                                for jt in jts:
                                    nc.tensor.matmul(
                                        cx_pair[u],
                                        v1[:, jt, h * (DK + 1) : (h + 1) * (DK + 1)],
                                        E_pair[u][:, jt],
                                        start=(jt == 0),
                                        stop=(jt == NJT - 1),
                                    )
                        # per-head: denominator -> normalize -> writeback
                        for u, h in enumerate(h_pair):
                            E, cx = E_pair[u], cx_pair[u]
                            cxu = att_sb.tile(
                                [DK + 1, IB], FP, tag="cxu", bufs=2, name=f"cxu_{h}"
                            )
                            nc.scalar.copy(cxu, cx)
                            r_row = att_sb.tile(
                                [1, IB], FP, tag="r", bufs=2, name=f"r_{h}"
                            )
                            nc.vector.reciprocal(r_row, cxu[DK : DK + 1, :])
                            R = att_ps.tile([P, IB], FP, tag="S", name=f"R_{h}")
                            nc.tensor.matmul(
                                R, ones_sb[:1, :P], r_row, start=True, stop=True
                            )
                            nc.vector.tensor_mul(
                                ctxT[
                                    (h % 2) * DK : (h % 2) * DK + DK,
                                    h // 2,
                                    ib * IB : (ib + 1) * IB,
                                ],
                                cxu[0:DK, :],
                                R[0:DK, :],
                            )
                            if skip_attn_write:
                                continue
                            if gps_chunks:
                                R_sb = att_sb.tile(
                                    [P, IB], FP, tag="Rsb", bufs=2, name=f"Rsb_{h}"
                                )
                                nc.scalar.copy(R_sb, R)
                            att_view = attnT_d.ap()[h].rearrange(
                                "(jt p) i -> p jt i", p=P
                            )
                            for cc in range(0, NJT, ACC):
                                w = min(ACC, NJT - cc)
                                A = att_sb.tile(
                                    [P, ACC, IB], FP, tag="A", bufs=3,
                                    name=f"A_{h}_{cc}",
                                )
                                if gps_chunks and (cc // ACC) < gps_chunks:
                                    nc.gpsimd.tensor_mul(
                                        A[:, :w],
                                        E[:, cc : cc + w],
                                        R_sb[:, None, :].broadcast_to([P, w, IB]),
                                    )
                                else:
                                    nc.vector.tensor_mul(
                                        A[:, :w],
                                        E[:, cc : cc + w],
                                        R[:, None, :].broadcast_to([P, w, IB]),
                                    )
                                nc.sync.dma_start(
                                    att_view[
                                        :, cc : cc + w, ib * IB : (ib + 1) * IB
                                    ],
                                    A[:, :w],
                                )
                    if fused_oproj and not skip_oproj:
                        oproj(range(ib * IT_PER_IB, (ib + 1) * IT_PER_IB))
                if not fused_oproj and not skip_oproj:
                    oproj(range(L // P))

    nc.finalize()
    return nc


def make_in_maps(x, W_qkv, b_qkv, W_o, L=L_FULL, D=D_FULL):
    """Shard full inputs into the 8 per-core input dicts."""
    x = np.asarray(x, dtype=np.float32)
    W_qkv = np.asarray(W_qkv, dtype=np.float32)
    b_qkv = np.asarray(b_qkv, dtype=np.float32)
    W_o = np.asarray(W_o, dtype=np.float32)
    VW = HPC * DK
    ones = np.ones([1, max(L, P)], np.float32)
    in_maps = []
    for c in range(N_CORES):
        b, g = divmod(c, HPC)
        s = VW * g
        in_maps.append(
            {
                "xT": np.ascontiguousarray(x[b].T),
                "wq": np.ascontiguousarray(W_qkv[:, s : s + VW]),
                "wk": np.ascontiguousarray(W_qkv[:, D + s : D + s + VW]),
                "wv": np.ascontiguousarray(W_qkv[:, 2 * D + s : 2 * D + s + VW]),
                "bq": np.ascontiguousarray(b_qkv[None, s : s + VW]),
                "bk": np.ascontiguousarray(b_qkv[None, D + s : D + s + VW]),
                "bv": np.ascontiguousarray(b_qkv[None, 2 * D + s : 2 * D + s + VW]),
                "wo": np.ascontiguousarray(W_o[s : s + VW, :]),
                "ones": ones,
            }
        )
    return in_maps


_NC_CACHE = {}


def get_nc(L=L_FULL, D=D_FULL, reps=1, **kw):
    key = (L, D, reps, tuple(sorted(kw.items())))
    if key not in _NC_CACHE:
        _NC_CACHE[key] = build_mha_core(L=L, D=D, reps=reps, **kw)
    return _NC_CACHE[key]


def assemble(results, b_o, B=2, L=L_FULL, D=D_FULL):
    b_o = np.asarray(b_o, dtype=np.float32)
    out = np.zeros([B, L, D], np.float32)
    attn = np.empty([B, H_TOTAL, L, L], np.float32)
    BL = 256  # cache-blocked un-transpose
    for c in range(len(results)):
        b, g = divmod(c, HPC)
        out[b] += results[c]["outp"]
        aT = results[c]["attnT"]
        for h in range(HPC):
            src, d = aT[h], attn[b, HPC * g + h]
            for i0 in range(0, L, BL):
                d[i0 : i0 + BL, :] = src[:, i0 : i0 + BL].T
    out += b_o
    return out, attn


def kernel(x, W_qkv, b_qkv, W_o, b_o):
    from concourse import bass_utils

    nc = get_nc()
    in_maps = make_in_maps(x, W_qkv, b_qkv, W_o)
    res = bass_utils.run_bass_kernel_spmd(nc, in_maps, list(range(N_CORES)))
    return assemble(res.results, b_o)
